# revision 36
# baseline (speedup 1.0000x reference)
"""Trainium2 Bass kernel for a dense pre-LN transformer block.

Problem: B=2, T=2048, C=1024, H=16 heads (d=64), FFN 4x, causal attention.

Parallelization over 8 NeuronCores (single SPMD program, one launch):
  - Attention: head-tensor-parallel. Core c computes heads {2c, 2c+1} for
    both batches: LN1 (replicated), Q/K/V projections, causal-block
    attention with unnormalized softmax (denominator via a ones-column in
    the value tile), reciprocal of the denominator computed sender-side.
  - FOUR AllToAlls (one per (batch, local-head)) redistribute attn^T from
    head-split to token-split; each overlaps the next attention unit or
    the early FFN work, so almost no collective time is exposed.
  - Post-A2A: core c owns tokens [256c, 256c+256) of BOTH batches:
    output projection + residual, LN2, FFN, residual.

Key implementation choices:
  - All [t,c] -> [c,t] transposes run on the DMA engines (xbar
    dma_start_transpose, bf16) instead of the PE: frees ~100us of PE time.
  - LayerNorm rsqrt = exp(-0.5*log(var+eps)) on the scalar engine so the
    whole kernel uses one activation table set (no table reload thrash).
  - g/beta of both LayerNorms are folded into the adjacent weight
    matrices host-side (bias rows enter via ones-row matmuls).
  - Softmax normalization: sender computes recip(den) (single-pass
    approx), the reciprocal rides the A2A as row 64; receiver applies it
    with one broadcast-DMA + multiply per batch (no expensive DVE
    reciprocal on broadcast data).
  - Causal masks multiply on GpSimd (otherwise idle), constants arrive in
    a handful of packed DMAs issued on the scalar queue so the x-tile DMAs
    lead the sync queue.
"""

import numpy as np
import ml_dtypes

B, T, C = 2, 2048, 1024
H, D = 16, 64
FF = 4 * C
EPS = 1e-5
NCORES = 8
TOK = 256   # tokens owned per core PER BATCH in the post-A2A phase
BT = B * T

_CACHE = {}
DEBUG = False


# --------------------------------------------------------------------------
# device program
# --------------------------------------------------------------------------
def _build_program():
    import concourse.bass as bass
    import concourse.mybir as mybir
    import concourse.tile as tile
    from concourse import bacc

    dt = mybir.dt
    f32 = dt.float32

    nc = bacc.Bacc("TRN2", target_bir_lowering=False, debug=False,
                   num_devices=NCORES)

    bf16 = dt.bfloat16
    x_full = nc.dram_tensor("x_full", [BT, C], f32, kind="ExternalInput")
    x_own = nc.dram_tensor("x_own", [2 * TOK, C], f32, kind="ExternalInput")
    wqkv = nc.dram_tensor("wqkv", [C, 384], bf16, kind="ExternalInput")
    cb = nc.dram_tensor("cb", [128, 35], f32, kind="ExternalInput")
    rows = nc.dram_tensor("rows", [1, 2176], bf16, kind="ExternalInput")
    masks = nc.dram_tensor("masks", [4, 128, 512], bf16, kind="ExternalInput")
    wproj = nc.dram_tensor("wproj", [C, C], bf16, kind="ExternalInput")
    w1 = nc.dram_tensor("w1", [C, FF], bf16, kind="ExternalInput")
    w2 = nc.dram_tensor("w2", [FF, C], bf16, kind="ExternalInput")
    out = nc.dram_tensor("out", [2 * TOK, C], f32, kind="ExternalOutput")
    if DEBUG:
        dh = nc.dram_tensor("dh", [128, 4 * C], bf16, kind="ExternalOutput")
        dhT = nc.dram_tensor("dhT", [128, 4 * 8 * 128], bf16,
                             kind="ExternalOutput")
        dq = nc.dram_tensor("dq", [128, T], bf16, kind="ExternalOutput")
        dk = nc.dram_tensor("dk", [128, T], bf16, kind="ExternalOutput")
        dva = nc.dram_tensor("dva", [128, 16 * 130], bf16, kind="ExternalOutput")
        da = nc.dram_tensor("da", [65, T], bf16, kind="ExternalOutput")
        din = nc.dram_tensor("din", [8, 65, TOK], bf16, kind="ExternalOutput")
        dout = nc.dram_tensor("dout", [8, 65, TOK], bf16, kind="ExternalOutput")
        dao = nc.dram_tensor("dao", [128, 8 * TOK], bf16, kind="ExternalOutput")
        dx2 = nc.dram_tensor("dx2", [128, 4 * C], f32, kind="ExternalOutput")
        dh2p = nc.dram_tensor("dh2p", [128, 8 * TOK], bf16, kind="ExternalOutput")
        df1 = nc.dram_tensor("df1", [128, 32 * 512], bf16, kind="ExternalOutput")

    with tile.TileContext(nc, num_cores=NCORES) as tc:
        _body(nc, tc, tile, mybir, bass, locals())
    nc.compile()
    return nc


def _body(nc, tc, tile, mybir, bass, io):
    dt = mybir.dt
    f32, f32r, bf16 = dt.float32, dt.float32r, dt.bfloat16
    AF = mybir.ActivationFunctionType
    OP = mybir.AluOpType

    x_full, x_own = io["x_full"], io["x_own"]
    wqkv, cb, rows, masks = io["wqkv"], io["cb"], io["rows"], io["masks"]
    wproj, w1, w2, out = io["wproj"], io["w1"], io["w2"], io["out"]

    # ---- persistent pools ----
    consts = tc.alloc_tile_pool(name="consts", bufs=1)
    persA = tc.alloc_tile_pool(name="persA", bufs=1)
    persD = tc.alloc_tile_pool(name="persD", bufs=1)
    dram = tc.alloc_tile_pool(name="dram", bufs=1, space="DRAM")

    # constants: few packed DMAs, issued on the scalar queue so that the
    # x-tile loads own the sync queue from t=0
    wqkv_sb = consts.tile([128, 8, 384], bf16, name="wqkv_sb")
    nc.scalar.dma_start(out=wqkv_sb[:],
                        in_=wqkv[:].rearrange("(cc p) d -> p cc d", p=128))
    cb_sb = consts.tile([128, 35], f32, name="cb_sb")
    nc.scalar.dma_start(out=cb_sb[:], in_=cb[:])
    rows_sb = consts.tile([1, 2176], bf16, name="rows_sb")
    nc.scalar.dma_start(out=rows_sb[:], in_=rows[:])
    mask_sb = consts.tile([128, 4, 512], bf16, name="mask_sb")
    nc.scalar.dma_start(out=mask_sb[:], in_=masks[:].rearrange("i p t -> p i t"))
    eps_sb = consts.tile([128, 1], f32, name="eps_sb")
    nc.vector.memset(eps_sb[:], EPS)
    onesf_sb = consts.tile([1, 128], f32, name="onesf_sb")
    nc.vector.memset(onesf_sb[:], 1.0)
    ones_b = rows_sb[0:1, 0:128]
    bproj_row = rows_sb[0:1, 128:1152]
    b2_row = rows_sb[0:1, 1152:2176]

    # phase-2 weight prefetched from t=0
    wp_sb = persD.tile([128, 8, C], bf16, name="wp_sb")
    nc.scalar.dma_start(out=wp_sb[:],
                        in_=wproj[:].rearrange("(dc p) e -> p dc e", p=128))

    # attention-persistent tensors
    qT = [persA.tile([128, T], bf16, name=f"qTb{b}") for b in range(2)]
    kT = [persA.tile([128, T], bf16, name=f"kTb{b}") for b in range(2)]
    vaug = [persA.tile([128, 16, 130], bf16, name=f"vaugb{b}") for b in range(2)]
    for b in range(2):
        nc.vector.memset(vaug[b][:, :, 64:65], 1.0)
        nc.vector.memset(vaug[b][:, :, 129:130], 1.0)

    a2a_in = [dram.tile([8, 65, TOK], bf16, name=f"a2a_in{k}") for k in range(4)]
    a2a_out = [dram.tile([8, 65, TOK], bf16, name=f"a2a_out{k}") for k in range(4)]

    w1r = w1[:].rearrange("(cc p) m -> p cc m", p=128)
    psQ = tc.alloc_tile_pool(name="psM", bufs=1, space="PSUM")

    # ======================================================================
    # Phase A: LN1 + Q/K/V per batch  (hT via DMA-xbar transpose)
    # ======================================================================
    with tc.tile_pool(name="lnq", bufs=1) as lnp:
        aT = [lnp.tile([65, T], bf16, tag="aT", bufs=2, name=f"aT{k}")
              for k in range(4)]
        for b in range(2):
            vT = lnp.tile([128, T], bf16, tag="vT", bufs=1, name=f"vT_{b}")
            with nc.named_scope(f"qkv_b{b}"):
                for tch in range(4):
                    h = lnp.tile([128, 4, C], bf16, tag="h", bufs=2,
                                 name=f"h_{b}_{tch}")
                    for ht in range(2):
                        row0 = b * T + tch * 512 + ht * 256
                        xt = lnp.tile([128, 2, C], f32, tag="xt", bufs=2,
                                      name=f"xt_{b}_{tch}_{ht}")
                        nc.sync.dma_start(
                            out=xt[:],
                            in_=x_full[row0:row0 + 256, :].rearrange(
                                "(s p) c -> p s c", p=128))
                        for s2 in range(2):
                            s = 2 * ht + s2
                            st = lnp.tile([128, 2, 6], f32, tag="st", bufs=2,
                                          name=f"st_{b}_{tch}_{s}")
                            nc.vector.bn_stats(out=st[:, 0, :],
                                               in_=xt[:, s2, 0:512])
                            nc.vector.bn_stats(out=st[:, 1, :],
                                               in_=xt[:, s2, 512:1024])
                            mv = lnp.tile([128, 2], f32, tag="mv", bufs=2,
                                          name=f"mv_{b}_{tch}_{s}")
                            nc.vector.bn_aggr(out=mv[:], in_=st[:])
                            lg = lnp.tile([128, 1], f32, tag="lg", bufs=2,
                                          name=f"lg_{b}_{tch}_{s}")
                            nc.scalar.activation(out=lg[:], in_=mv[:, 1:2],
                                                 func=AF.Ln, bias=eps_sb[:])
                            rs = lnp.tile([128, 1], f32, tag="rs", bufs=2,
                                          name=f"rs_{b}_{tch}_{s}")
                            nc.scalar.activation(out=rs[:], in_=lg[:],
                                                 func=AF.Exp, scale=-0.5)
                            nc.vector.tensor_scalar(out=h[:, s, :],
                                                    in0=xt[:, s2, :],
                                                    scalar1=mv[:, 0:1],
                                                    scalar2=rs[:],
                                                    op0=OP.subtract, op1=OP.mult)
                    # hT chunks via xbar DMA transpose (contiguous dst per s)
                    hTb = lnp.tile([128, 4, 8, 128], bf16, tag="hTb", bufs=2,
                                   name=f"hTb_{b}_{tch}")
                    for s in range(4):
                        nc.sync.dma_start_transpose(out=hTb[:, s], in_=h[:, s, :])
                    if DEBUG and b == 0 and tch == 0:
                        nc.sync.dma_start(
                            out=io["dh"][:],
                            in_=h[:].rearrange("p a b -> p (a b)"))
                        nc.sync.dma_start(
                            out=io["dhT"][:],
                            in_=hTb[:].rearrange("p a b c -> p (a b c)"))
                    col = tch * 512
                    # q^T, k^T, v^T: N=128 matmuls, stationary reused over
                    # the 4 token sub-chunks (xbar output is 128-contiguous)
                    for wi, dst in ((0, qT[b]), (1, kT[b]), (2, vT)):
                        pqk = psQ.tile([128, 512], f32, tag="pqv", bufs=2,
                                       name=f"pq_{b}_{tch}_{wi}")
                        for s in range(4):
                            for cc in range(8):
                                nc.tensor.matmul(
                                    pqk[:, s * 128:(s + 1) * 128],
                                    wqkv_sb[:, cc, wi * 128:(wi + 1) * 128],
                                    hTb[:, s, cc, :],
                                    start=(cc == 0), stop=(cc == 7))
                        nc.vector.tensor_scalar_add(out=dst[:, col:col + 512],
                                                    in0=pqk[:],
                                                    scalar1=cb_sb[:, wi:wi + 1])
                # v back to [t, d] layout: one full-tile xbar transpose,
                # then split into the per-head [.. 65 ..] stationary layout
                vfull = lnp.tile([128, 16, 128], bf16, tag="vfull", bufs=1,
                                 name=f"vfull_{b}")
                nc.sync.dma_start_transpose(out=vfull[:], in_=vT[:])
                nc.vector.tensor_copy(out=vaug[b][:, :, 0:64],
                                      in_=vfull[:, :, 0:64])
                nc.vector.tensor_copy(out=vaug[b][:, :, 65:129],
                                      in_=vfull[:, :, 64:128])

        # ==================================================================
        # Phase B: causal attention per (batch, local head) + its A2A
        # ==================================================================
        for b in range(2):
            for hh in range(2):
                k4 = 2 * b + hh
                hp = 64 * hh
                vs = 65 * hh
                with nc.named_scope(f"attn_b{b}h{hh}"):
                    for half in range(2):
                        qc0 = half * 1024
                        pat = [psQ.tile([128, 512], f32, tag=f"pat{i}", bufs=1,
                                        name=f"pat_{k4}_{half}_{i}")
                               for i in range(2)]
                        nsb = 8 * half + 8
                        for sb in range(nsb):
                            act0 = 0 if sb < 8 * half + 4 else 1
                            dtc = sb // 4 - 2 * half
                            ps = psQ.tile([128, 1024], f32, tag="ps", bufs=2,
                                          name=f"ps_{k4}_{half}_{sb}")
                            for i in range(act0, 2):
                                nc.tensor.matmul(
                                    ps[:, i * 512:(i + 1) * 512],
                                    kT[b][hp:hp + 64, sb * 128:sb * 128 + 128],
                                    qT[b][hp:hp + 64,
                                          qc0 + i * 512:qc0 + (i + 1) * 512],
                                    start=True, stop=True)
                            pt = lnp.tile([128, 1024], bf16, tag="pt", bufs=3,
                                          name=f"pt_{k4}_{half}_{sb}")
                            nc.scalar.activation(out=pt[:, act0 * 512:1024],
                                                 in_=ps[:, act0 * 512:1024],
                                                 func=AF.Exp, scale=0.125)
                            if dtc >= act0:
                                nc.gpsimd.tensor_mul(
                                    pt[:, dtc * 512:(dtc + 1) * 512],
                                    pt[:, dtc * 512:(dtc + 1) * 512],
                                    mask_sb[:, sb % 4, :])
                            for i in range(act0, 2):
                                last = 8 * half + 3 if i == 0 else nsb - 1
                                nc.tensor.matmul(
                                    pat[i][0:65, :], vaug[b][:, sb, vs:vs + 65],
                                    pt[:, i * 512:(i + 1) * 512],
                                    start=(sb == 0), stop=(sb == last))
                        for i in range(2):
                            qcol = (2 * half + i) * 512
                            nc.vector.tensor_copy(
                                out=aT[k4][64:65, qcol:qcol + 512],
                                in_=pat[i][64:65, :])
                            nc.vector.tensor_copy(
                                out=aT[k4][0:64, qcol:qcol + 512],
                                in_=pat[i][0:64, :])
                if DEBUG and k4 == 0:
                    nc.sync.dma_start(out=io["dq"][:], in_=qT[0][:])
                    nc.sync.dma_start(out=io["dk"][:], in_=kT[0][:])
                    nc.sync.dma_start(
                        out=io["dva"][:],
                        in_=vaug[0][:].rearrange("p a b -> p (a b)"))
                    nc.sync.dma_start(out=io["da"][:], in_=aT[0][:])
                # ship: shard j = tokens [256j, 256j+256) of this (b, head)
                nc.sync.dma_start(
                    out=a2a_in[k4][:].rearrange("j d t -> d j t"),
                    in_=aT[k4][:].rearrange("d (j t) -> d j t", j=8))
                nc.gpsimd.collective_compute(
                    "AllToAll", mybir.AluOpType.bypass,
                    replica_groups=[list(range(NCORES))],
                    ins=[a2a_in[k4][:].opt()], outs=[a2a_out[k4][:].opt()])
                if DEBUG and k4 == 0:
                    nc.sync.dma_start(out=io["din"][:], in_=a2a_in[0][:])
                    nc.sync.dma_start(out=io["dout"][:], in_=a2a_out[0][:])

    # ======================================================================
    # Phase C: per batch: unpack A2A + normalize + proj + LN2 + FFN1
    # ======================================================================
    x2 = persD.tile([128, 4, C], f32, name="x2")
    ff1T = persD.tile([128, 32, 512], bf16, name="ff1T")
    h2T = [persD.tile([128, 2, 8, 128], bf16, name=f"h2T_{b}") for b in range(2)]

    for b in range(2):
        tcol = TOK * b
        with tc.tile_pool(name=f"prj{b}", bufs=1) as prp:
            xo = prp.tile([128, 2, C], f32, tag="xo", name=f"xo_{b}")
            nc.scalar.dma_start(
                out=xo[:],
                in_=x_own[TOK * b:TOK * (b + 1), :].rearrange(
                    "(q p) c -> p q c", p=128))
            aT_own = prp.tile([128, 8, TOK], bf16, tag="aTo", name=f"aTo_{b}")
            h2p = prp.tile([128, 8, TOK], bf16, tag="h2p", name=f"h2p_{b}")
            rb = prp.tile([128, 8, TOK], bf16, tag="rb", name=f"rb_{b}")
            for hh in range(2):
                k4 = 2 * b + hh
                nc.sync.dma_start(
                    out=aT_own[64 * hh:64 * hh + 64, :, :],
                    in_=a2a_out[k4][:, 0:64, :].rearrange("r d t -> d r t"))
                for r in range(8):
                    nc.sync.dma_start(
                        out=rb[64 * hh:64 * hh + 64, r, :],
                        in_=a2a_out[k4][r, 64:65, :].to_broadcast([64, TOK]))
            for r in range(8):
                rbf = prp.tile([128, TOK], f32, tag="rbf", bufs=2,
                               name=f"rbf_{b}_{r}")
                nc.vector.tensor_copy(out=rbf[:], in_=rb[:, r, :])
                rbf2 = prp.tile([128, TOK], f32, tag="rbf2", bufs=2,
                                name=f"rbf2_{b}_{r}")
                nc.vector.reciprocal_approx_fast(out=rbf2[:], in_=rbf[:])
                nc.vector.tensor_mul(aT_own[:, r, :], aT_own[:, r, :],
                                     rbf2[:])
            if DEBUG and b == 0:
                nc.sync.dma_start(
                    out=io["dao"][:],
                    in_=aT_own[:].rearrange("p a b -> p (a b)"))
            h2 = prp.tile([128, 2, C], bf16, tag="h2", name=f"h2_{b}")
            with nc.named_scope(f"proj_ln2_b{b}"):
                for tqi in range(2):
                    tq = 2 * b + tqi
                    for eh in range(2):
                        pp = psQ.tile([128, 512], f32, tag="pqv", bufs=2,
                                      name=f"pp_{tq}_{eh}")
                        for dc in range(8):
                            nc.tensor.matmul(
                                pp[:], aT_own[:, dc, tqi * 128:(tqi + 1) * 128],
                                wp_sb[:, dc, eh * 512:eh * 512 + 512],
                                start=(dc == 0), stop=False)
                        nc.tensor.matmul(pp[:], ones_b,
                                         bproj_row[0:1, eh * 512:eh * 512 + 512],
                                         start=False, stop=True)
                        nc.vector.tensor_add(x2[:, tq, eh * 512:eh * 512 + 512],
                                             pp[:], xo[:, tqi, eh * 512:eh * 512 + 512])
                    st2 = prp.tile([128, 2, 6], f32, tag="st2", bufs=2,
                                   name=f"st2_{tq}")
                    nc.vector.bn_stats(out=st2[:, 0, :], in_=x2[:, tq, 0:512])
                    nc.vector.bn_stats(out=st2[:, 1, :], in_=x2[:, tq, 512:1024])
                    mv2 = prp.tile([128, 2], f32, tag="mv2", bufs=2,
                                   name=f"mv2_{tq}")
                    nc.vector.bn_aggr(out=mv2[:], in_=st2[:])
                    lg2 = prp.tile([128, 1], f32, tag="lg2", bufs=2,
                                   name=f"lg2_{tq}")
                    nc.scalar.activation(out=lg2[:], in_=mv2[:, 1:2],
                                         func=AF.Ln, bias=eps_sb[:])
                    rs2 = prp.tile([128, 1], f32, tag="rs2", bufs=2,
                                   name=f"rs2_{tq}")
                    nc.scalar.activation(out=rs2[:], in_=lg2[:],
                                         func=AF.Exp, scale=-0.5)
                    nc.vector.tensor_scalar(out=h2[:, tqi, :], in0=x2[:, tq, :],
                                            scalar1=mv2[:, 0:1], scalar2=rs2[:],
                                            op0=OP.subtract, op1=OP.mult)
                    nc.sync.dma_start_transpose(out=h2T[b][:, tqi],
                                                in_=h2[:, tqi, :])
                # repack [tq, cc, 128] -> [cc, 256] (contiguous moving
                # operand for FFN1) on the otherwise idle GpSimd engine
                for tqi in range(2):
                    nc.gpsimd.tensor_copy(
                        out=h2p[:, :, tqi * 128:(tqi + 1) * 128],
                        in_=h2T[b][:, tqi])
            with nc.named_scope(f"ffn1_b{b}"):
                for w in range(16):
                    w1w = prp.tile([128, 8, 256], bf16, tag="w1w", bufs=3,
                                   name=f"w1w_{b}_{w}")
                    nc.scalar.dma_start(out=w1w[:],
                                        in_=w1r[:, :, w * 256:(w + 1) * 256])
                    for m2 in range(2):
                        m = w * 2 + m2
                        pf = psQ.tile([128, 512], f32, tag="pqv", bufs=2,
                                      name=f"pf_{b}_{m}")
                        for cc in range(8):
                            nc.tensor.matmul(
                                pf[:, 0:256], w1w[:, cc, m2 * 128:(m2 + 1) * 128],
                                h2p[:, cc, :],
                                start=(cc == 0), stop=(cc == 7))
                        nc.scalar.activation(out=ff1T[:, m, tcol:tcol + TOK],
                                             in_=pf[:, 0:256], func=AF.Relu,
                                             bias=cb_sb[:, 3 + m:4 + m])
            if DEBUG and b == 0:
                nc.sync.dma_start(
                    out=io["dh2p"][:],
                    in_=h2p[:].rearrange("p a b -> p (a b)"))
    if DEBUG:
        nc.sync.dma_start(out=io["dx2"][:],
                          in_=x2[:].rearrange("p a b -> p (a b)"))
        nc.sync.dma_start(out=io["df1"][:],
                          in_=ff1T[:].rearrange("p a b -> p (a b)"))

    psQ.release()

    # ======================================================================
    # Phase D: FFN2 over all 4 token-quarters, single w2 stream
    # ======================================================================
    with tc.tile_pool(name="ffn2p", bufs=1) as f2p, \
         tc.tile_pool(name="ps2", bufs=1, space="PSUM") as ps2p, \
         nc.named_scope("ffn2"):
        pso = [ps2p.tile([128, C], f32, tag="pso", bufs=4, name=f"pso_{tq}")
               for tq in range(4)]
        for mc in range(32):
            w2t = f2p.tile([128, C], bf16, tag="w2t", bufs=6, name=f"w2t_{mc}")
            nc.scalar.dma_start(out=w2t[:], in_=w2[mc * 128:(mc + 1) * 128, :])
            for tq in range(4):
                for eh in range(2):
                    nc.tensor.matmul(pso[tq][:, eh * 512:(eh + 1) * 512],
                                     ff1T[:, mc, tq * 128:(tq + 1) * 128],
                                     w2t[:, eh * 512:(eh + 1) * 512],
                                     start=(mc == 0), stop=False)
        for tq in range(4):
            for eh in range(2):
                nc.tensor.matmul(pso[tq][:, eh * 512:(eh + 1) * 512],
                                 ones_b, b2_row[0:1, eh * 512:(eh + 1) * 512],
                                 start=False, stop=True)
            ot = f2p.tile([128, C], f32, tag="ot", bufs=2, name=f"ot_{tq}")
            nc.vector.tensor_add(ot[:], pso[tq][:], x2[:, tq, :])
            nc.scalar.dma_start(out=out[tq * 128:(tq + 1) * 128, :], in_=ot[:])
    persD.release()
    persA.release()
    consts.release()
    dram.release()


# --------------------------------------------------------------------------
# host driver
# --------------------------------------------------------------------------
def _make_in_maps(inputs):
    bf = ml_dtypes.bfloat16
    x = np.ascontiguousarray(np.asarray(inputs["x"], np.float32))
    wq = np.asarray(inputs["wq"], np.float32)
    wk = np.asarray(inputs["wk"], np.float32)
    wv = np.asarray(inputs["wv"], np.float32)
    w_proj = np.asarray(inputs["w_proj"], np.float32)
    b_proj = np.asarray(inputs["b_proj"], np.float32)
    w1 = np.asarray(inputs["w1"], np.float32)
    b1 = np.asarray(inputs["b1"], np.float32)
    w2 = np.asarray(inputs["w2"], np.float32)
    b2 = np.asarray(inputs["b2"], np.float32)
    g1 = np.asarray(inputs["g1"], np.float32)
    be1 = np.asarray(inputs["be1"], np.float32)
    g2 = np.asarray(inputs["g2"], np.float32)
    be2 = np.asarray(inputs["be2"], np.float32)

    xf = x.reshape(BT, C)
    i_mask = np.zeros((4, 128, 512), np.float32)
    s_idx = np.arange(128)[:, None]
    t_idx = np.arange(512)[None, :]
    for i in range(4):
        i_mask[i] = (s_idx + 128 * i <= t_idx).astype(np.float32)

    w1f = g2[:, None] * w1                       # fold LN2 gamma
    b1f = b1 + be2 @ w1                          # fold LN2 beta
    rows = np.concatenate([np.ones(128, np.float32), b_proj, b2])[None, :]

    common = dict(
        x_full=xf,
        masks=i_mask.astype(bf),
        rows=rows.astype(bf),
        wproj=np.ascontiguousarray(w_proj).astype(bf),
        w1=np.ascontiguousarray(w1f).astype(bf),
        w2=np.ascontiguousarray(w2).astype(bf),
    )
    in_maps = []
    for c in range(NCORES):
        wqp = np.concatenate([wq[2 * c], wq[2 * c + 1]], axis=1)  # [C,128]
        wkp = np.concatenate([wk[2 * c], wk[2 * c + 1]], axis=1)
        wvp = np.concatenate([wv[2 * c], wv[2 * c + 1]], axis=1)
        wqkv = np.concatenate([g1[:, None] * wqp, g1[:, None] * wkp,
                               g1[:, None] * wvp], axis=1)        # [C,384]
        cbm = np.zeros((128, 35), np.float32)
        cbm[:, 0] = be1 @ wqp
        cbm[:, 1] = be1 @ wkp
        cbm[:, 2] = be1 @ wvp
        cbm[:, 3:35] = np.ascontiguousarray(b1f.reshape(32, 128).T)
        m = dict(common)
        m["wqkv"] = np.ascontiguousarray(wqkv).astype(bf)
        m["cb"] = np.ascontiguousarray(cbm)
        m["x_own"] = np.ascontiguousarray(
            np.concatenate([xf[TOK * c:TOK * (c + 1)],
                            xf[T + TOK * c:T + TOK * (c + 1)]], axis=0))
        in_maps.append(m)
    return in_maps


LAST_RESULTS = None


def kernel(trace=False, **inputs):
    global LAST_RESULTS
    from concourse import bass_utils

    if "nc" not in _CACHE:
        _CACHE["nc"] = _build_program()
    nc = _CACHE["nc"]
    in_maps = _make_in_maps(inputs)
    res = bass_utils.run_bass_kernel_spmd(
        nc, in_maps, core_ids=list(range(NCORES)), trace=trace)
    LAST_RESULTS = res
    out = np.zeros((B, T, C), np.float32)
    for c in range(NCORES):
        r = res.results[c]["out"]
        out[0, TOK * c:TOK * (c + 1), :] = r[0:TOK]
        out[1, TOK * c:TOK * (c + 1), :] = r[TOK:2 * TOK]
    return out
